# revision 7
# baseline (speedup 1.0000x reference)
"""Trainium2 Bass kernel for nn_COS_Loss_45423574122758.

The reference crops (8,3,1024,1024) inputs to a 7x7 grid of 128x128
windows and computes per-window sums of x*t, x*x, t*t reduced over
batch+channel+window, then a cosine per window — but the final output
only reads cos[-1,-1]: the window at rows 768:896, cols 768:896. So the
scalar output depends only on the (8,3,128,128) last-window slice of
each input.

Strategy: shard that slice by batch across the 8 NeuronCores (one batch
per core). Each core DMAs its (3,128,128) slice pair viewed as
(128,384), computes per-partition partial sums of x*t, x*x, t*t on the
vector engine, and DMAs out a (128,3) stats tile. The host sums the
8x128x3 partials and finishes the scalar cosine math.

Raw single-engine bass (no TileContext): every instruction lives on the
vector-engine queue, so there is no cross-engine semaphore traffic and
no Tile drain/barrier tail.
"""

import numpy as np

import concourse.bass as bass
from concourse import bacc, mybir
from concourse.bass_utils import run_bass_kernel_spmd

_K = 128          # sliding window size
_R0 = 768         # last window start: (ceil((1024-128)/128) - 1) * 128
_B = 8
_NPART = 128      # SBUF partitions
_NFREE = 384      # 3 channels * 128 cols per partition row
_COUNT = 49.0     # 7*7 windows

# Set by test.py to capture a neuron-profile trace; harness leaves it off.
PROFILE = False
LAST_EXEC_TIME_NS = None

_cached = {}


def _program() -> bass.Bass:
    if "nc" in _cached:
        return _cached["nc"]

    f32 = mybir.dt.float32
    nc = bacc.Bacc(
        trn_type="TRN2",
        target_bir_lowering=False,
        debug=False,
        num_devices=_B,
    )
    x_d = nc.dram_tensor("x", [_NPART, _NFREE], f32, kind="ExternalInput").ap()
    t_d = nc.dram_tensor("t", [_NPART, _NFREE], f32, kind="ExternalInput").ap()
    s_d = nc.dram_tensor("stats", [_NPART, 3], f32, kind="ExternalOutput").ap()

    X = nc.alloc_sbuf_tensor("X", [_NPART, _NFREE], f32).ap()
    T = nc.alloc_sbuf_tensor("T", [_NPART, _NFREE], f32).ap()
    P = nc.alloc_sbuf_tensor("P", [_NPART, 3, _NFREE], f32).ap()
    S = nc.alloc_sbuf_tensor("S", [_NPART, 3], f32).ap()

    with (
        nc.Block() as block,
        nc.semaphore("dsem") as dsem,
        nc.semaphore("vsem") as vsem,
    ):

        @block.sync
        def _(sp: bass.BassEngine):
            sp.dma_start(out=X, in_=x_d).then_inc(dsem, 16)
            sp.dma_start(out=T, in_=t_d).then_inc(dsem, 16)
            sp.wait_ge(vsem, 1)
            sp.dma_start(out=s_d, in_=S).then_inc(dsem, 16)
            sp.wait_ge(dsem, 48)

        @block.vector
        def _(v: bass.BassEngine):
            v.wait_ge(dsem, 32)
            v.tensor_mul(P[:, 0], X, T)
            v.tensor_mul(P[:, 1], X, X)
            v.tensor_mul(P[:, 2], T, T)
            v.reduce_sum(S, P, axis=mybir.AxisListType.X).then_inc(vsem, 1)

    nc.compile()
    _cached["nc"] = nc
    return nc


def kernel(input: np.ndarray, target: np.ndarray) -> np.ndarray:
    global LAST_EXEC_TIME_NS
    inp = np.asarray(input, dtype=np.float32)
    tar = np.asarray(target, dtype=np.float32)

    xs = inp[:, :, _R0:_R0 + _K, _R0:_R0 + _K]  # (8,3,128,128)
    ts = tar[:, :, _R0:_R0 + _K, _R0:_R0 + _K]
    in_maps = [
        {
            "x": np.ascontiguousarray(xs[b]).reshape(_NPART, _NFREE),
            "t": np.ascontiguousarray(ts[b]).reshape(_NPART, _NFREE),
        }
        for b in range(_B)
    ]

    nc = _program()
    res = run_bass_kernel_spmd(nc, in_maps, core_ids=list(range(_B)),
                               trace=PROFILE)
    LAST_EXEC_TIME_NS = res.exec_time_ns

    stats = np.stack([res.results[b]["stats"] for b in range(_B)])  # (8,128,3)
    dot, ni, nt = stats.astype(np.float64).sum(axis=(0, 1))
    cos = dot / (np.sqrt(ni) * np.sqrt(nt))
    return np.array((cos - 1.0) ** 2 / _COUNT, dtype=np.float32)


# revision 8
# speedup vs baseline: 1.2620x; 1.2620x over previous
"""Trainium2 Bass kernel for nn_COS_Loss_45423574122758.

The reference crops (8,3,1024,1024) inputs to a 7x7 grid of 128x128
windows and computes per-window sums of x*t, x*x, t*t reduced over
batch+channel+window, then a cosine per window — but the final output
only reads cos[-1,-1]: the window at rows 768:896, cols 768:896. So the
scalar output depends only on the (8,3,128,128) last-window slice of
each input.

Strategy: shard that slice by batch across the 8 NeuronCores (one batch
per core). Each core DMAs its (3,128,128) slice pair viewed as
(128,384), computes per-partition partial sums of x*t, x*x, t*t on the
vector engine, and DMAs out a (128,3) stats tile. The host sums the
8x128x3 partials and finishes the scalar cosine math.

Raw single-engine bass (no TileContext): every instruction lives on the
vector-engine queue, so there is no cross-engine semaphore traffic and
no Tile drain/barrier tail.
"""

import numpy as np

import concourse.bass as bass
from concourse import bacc, mybir
from concourse.bass_utils import run_bass_kernel_spmd

_K = 128          # sliding window size
_R0 = 768         # last window start: (ceil((1024-128)/128) - 1) * 128
_B = 8
_NPART = 128      # SBUF partitions
_NFREE = 384      # 3 channels * 128 cols per partition row
_COUNT = 49.0     # 7*7 windows

# Set by test.py to capture a neuron-profile trace; harness leaves it off.
PROFILE = False
LAST_EXEC_TIME_NS = None

_cached = {}


def _program() -> bass.Bass:
    if "nc" in _cached:
        return _cached["nc"]

    f32 = mybir.dt.float32
    nc = bacc.Bacc(
        trn_type="TRN2",
        target_bir_lowering=False,
        debug=False,
        num_devices=_B,
    )
    x_d = nc.dram_tensor("x", [_NPART, _NFREE], f32, kind="ExternalInput").ap()
    t_d = nc.dram_tensor("t", [_NPART, _NFREE], f32, kind="ExternalInput").ap()
    s_d = nc.dram_tensor("stats", [_NPART, 3], f32, kind="ExternalOutput").ap()

    X = nc.alloc_sbuf_tensor("X", [_NPART, _NFREE], f32).ap()
    T = nc.alloc_sbuf_tensor("T", [_NPART, _NFREE], f32).ap()
    P = nc.alloc_sbuf_tensor("P", [_NPART, 2, _NFREE], f32).ap()
    PV = nc.alloc_sbuf_tensor("PV", [_NPART, _NFREE], f32).ap()
    S = nc.alloc_sbuf_tensor("S", [_NPART, 3], f32).ap()

    mult = mybir.AluOpType.mult

    with (
        nc.Block() as block,
        nc.semaphore("xsem") as xsem,
        nc.semaphore("tsem") as tsem,
        nc.semaphore("vsem") as vsem,
        nc.semaphore("ssem") as ssem,
        nc.semaphore("osem") as osem,
    ):

        @block.sync
        def _(sp: bass.BassEngine):
            sp.dma_start(out=X, in_=x_d).then_inc(xsem, 16)
            sp.wait_ge(vsem, 1)
            sp.wait_ge(ssem, 1)
            sp.dma_start(out=s_d, in_=S).then_inc(osem, 16)
            sp.wait_ge(osem, 16)

        @block.scalar
        def _(act: bass.BassEngine):
            act.dma_start(out=T, in_=t_d).then_inc(tsem, 16)
            act.wait_ge(xsem, 16)
            act.activation(P[:, 0], X, mybir.ActivationFunctionType.Square,
                           accum_out=S[:, 1:2])
            act.wait_ge(tsem, 16)
            act.activation(P[:, 1], T, mybir.ActivationFunctionType.Square,
                           accum_out=S[:, 2:3]).then_inc(ssem, 1)

        @block.vector
        def _(v: bass.BassEngine):
            v.wait_ge(xsem, 16)
            v.wait_ge(tsem, 16)
            v.scalar_tensor_tensor(PV, X, 1.0, T, op0=mult, op1=mult,
                                   accum_out=S[:, 0:1]).then_inc(vsem, 1)

    nc.compile()
    _cached["nc"] = nc
    return nc


def kernel(input: np.ndarray, target: np.ndarray) -> np.ndarray:
    global LAST_EXEC_TIME_NS
    inp = np.asarray(input, dtype=np.float32)
    tar = np.asarray(target, dtype=np.float32)

    xs = inp[:, :, _R0:_R0 + _K, _R0:_R0 + _K]  # (8,3,128,128)
    ts = tar[:, :, _R0:_R0 + _K, _R0:_R0 + _K]
    in_maps = [
        {
            "x": np.ascontiguousarray(xs[b]).reshape(_NPART, _NFREE),
            "t": np.ascontiguousarray(ts[b]).reshape(_NPART, _NFREE),
        }
        for b in range(_B)
    ]

    nc = _program()
    res = run_bass_kernel_spmd(nc, in_maps, core_ids=list(range(_B)),
                               trace=PROFILE)
    LAST_EXEC_TIME_NS = res.exec_time_ns

    stats = np.stack([res.results[b]["stats"] for b in range(_B)])  # (8,128,3)
    dot, ni, nt = stats.astype(np.float64).sum(axis=(0, 1))
    cos = dot / (np.sqrt(ni) * np.sqrt(nt))
    return np.array((cos - 1.0) ** 2 / _COUNT, dtype=np.float32)


# revision 11
# speedup vs baseline: 1.4378x; 1.1392x over previous
"""Trainium2 Bass kernel for nn_COS_Loss_45423574122758.

The reference crops (8,3,1024,1024) inputs to a 7x7 grid of 128x128
windows and computes per-window sums of x*t, x*x, t*t reduced over
batch+channel+window, then a cosine per window — but the final output
only reads cos[-1,-1]: the window at rows 768:896, cols 768:896. So the
scalar output depends only on the (8,3,128,128) last-window slice of
each input.

Strategy: shard that slice by batch across the 8 NeuronCores (one batch
per core). Each core DMAs its (3,128,128) slice pair viewed as
(128,384), computes per-partition partial sums of x*t, x*x, t*t on the
vector engine, and DMAs out a (128,3) stats tile. The host sums the
8x128x3 partials and finishes the scalar cosine math.

Raw single-engine bass (no TileContext): every instruction lives on the
vector-engine queue, so there is no cross-engine semaphore traffic and
no Tile drain/barrier tail.
"""

import numpy as np

try:  # persistent XLA cache: lets a fresh process skip the neuronx compile
    import jax

    jax.config.update("jax_compilation_cache_dir", "/tmp/jax_cache_cosloss")
    jax.config.update("jax_persistent_cache_min_entry_size_bytes", -1)
    jax.config.update("jax_persistent_cache_min_compile_time_secs", 0)
except Exception:
    pass

import concourse.bass as bass
from concourse import bacc, mybir
from concourse.bass_utils import run_bass_kernel_spmd

_K = 128          # sliding window size
_R0 = 768         # last window start: (ceil((1024-128)/128) - 1) * 128
_B = 8
_NPART = 128      # SBUF partitions
_NFREE = 384      # 3 channels * 128 cols per partition row
_COUNT = 49.0     # 7*7 windows

# Set by test.py to capture a neuron-profile trace; harness leaves it off.
PROFILE = False
LAST_EXEC_TIME_NS = None

_cached = {}


def _program() -> bass.Bass:
    if "nc" in _cached:
        return _cached["nc"]

    f32 = mybir.dt.float32
    nc = bacc.Bacc(
        trn_type="TRN2",
        target_bir_lowering=False,
        debug=False,
        num_devices=_B,
    )
    x_d = nc.dram_tensor("x", [_NPART, _NFREE], f32, kind="ExternalInput").ap()
    t_d = nc.dram_tensor("t", [_NPART, _NFREE], f32, kind="ExternalInput").ap()
    s_d = nc.dram_tensor("stats", [_NPART, 3], f32, kind="ExternalOutput").ap()

    X = nc.alloc_sbuf_tensor("X", [_NPART, _NFREE], f32).ap()
    T = nc.alloc_sbuf_tensor("T", [_NPART, _NFREE], f32).ap()
    PV = nc.alloc_sbuf_tensor("PV", [_NPART, _NFREE], f32).ap()
    S = nc.alloc_sbuf_tensor("S", [_NPART, 3], f32).ap()

    mult = mybir.AluOpType.mult

    with (
        nc.Block() as block,
        nc.semaphore("xsem") as xsem,
        nc.semaphore("tsem") as tsem,
        nc.semaphore("vsem") as vsem,
        nc.semaphore("osem") as osem,
    ):

        @block.sync
        def _(sp: bass.BassEngine):
            sp.dma_start(out=X, in_=x_d).then_inc(xsem, 16)
            sp.wait_ge(vsem, 1)
            sp.dma_start(out=s_d, in_=S).then_inc(osem, 16)

        @block.scalar
        def _(act: bass.BassEngine):
            act.dma_start(out=T, in_=t_d).then_inc(tsem, 16)

        @block.vector
        def _(v: bass.BassEngine):
            v.wait_ge(xsem, 16)
            v.scalar_tensor_tensor(PV, X, 1.0, X, op0=mult, op1=mult,
                                   accum_out=S[:, 1:2])
            v.wait_ge(tsem, 16)
            v.scalar_tensor_tensor(PV, X, 1.0, T, op0=mult, op1=mult,
                                   accum_out=S[:, 0:1])
            v.scalar_tensor_tensor(PV, T, 1.0, T, op0=mult, op1=mult,
                                   accum_out=S[:, 2:3]).then_inc(vsem, 1)

    nc.compile()
    _cached["nc"] = nc
    return nc


def kernel(input: np.ndarray, target: np.ndarray) -> np.ndarray:
    global LAST_EXEC_TIME_NS
    inp = np.asarray(input, dtype=np.float32)
    tar = np.asarray(target, dtype=np.float32)

    xs = inp[:, :, _R0:_R0 + _K, _R0:_R0 + _K]  # (8,3,128,128)
    ts = tar[:, :, _R0:_R0 + _K, _R0:_R0 + _K]
    in_maps = [
        {
            "x": np.ascontiguousarray(xs[b]).reshape(_NPART, _NFREE),
            "t": np.ascontiguousarray(ts[b]).reshape(_NPART, _NFREE),
        }
        for b in range(_B)
    ]

    nc = _program()
    res = run_bass_kernel_spmd(nc, in_maps, core_ids=list(range(_B)),
                               trace=PROFILE)
    LAST_EXEC_TIME_NS = res.exec_time_ns

    stats = np.stack([res.results[b]["stats"] for b in range(_B)])  # (8,128,3)
    dot, ni, nt = stats.astype(np.float64).sum(axis=(0, 1))
    cos = dot / (np.sqrt(ni) * np.sqrt(nt))
    return np.array((cos - 1.0) ** 2 / _COUNT, dtype=np.float32)
